# revision 1
# baseline (speedup 1.0000x reference)
"""VQ codebook kernel for 8 TRN2 NeuronCores.

reference math:
    dist  = ||z||^2 + ||e||^2 - 2 z.e       (argmin over 8192 codes)
    out   = (codebook[idx] (B,S,D) f32, idx (B,S,1) i32)

device math (argmin-equivalent, ||z||^2 dropped):
    score = z.e - ||e||^2/2                 (argmax)

Sharding: data-parallel on the flattened token dim (32768 tokens -> 4096
per core), codebook replicated.

Precision: the z.e GEMM runs as three bf16 passes (zh.eh + zh.el + zl.eh,
hi/lo bf16 limb split) which reproduces the fp32 reference argmin exactly
on this data (validated: 0/32768 flips; fp32r single-pass flips 8 tokens).
The -||e||^2/2 bias is folded into the same PSUM accumulation as a K=3
matmul of three bf16 bias limbs against a ones vector.

Argmax: per 2048-code PSUM group, DVE max (top-8) + max_index directly on
PSUM; the 4 group winners are combined with first-occurrence tie semantics
matching jnp.argmin. Embedding rows are gathered from DRAM by indirect DMA.
"""

import sys

sys.path.insert(0, "/opt/trn_rl_repo")

import numpy as np
import ml_dtypes

B, S, D, K = 8, 4096, 256, 8192
NCORES = 8
NTOK = B * S
TOK_PER_CORE = NTOK // NCORES
P = 128


def _build(n_tok, n_codes, gsize):
    import concourse.bacc as bacc
    import concourse.bass as bass
    import concourse.mybir as mybir
    import concourse.tile as tile

    dt = mybir.dt
    n_tiles = n_tok // P
    n_groups = n_codes // gsize
    nblk = gsize // 512
    assert n_groups <= 8

    nc = bacc.Bacc(trn_type="TRN2")
    zh = nc.dram_tensor("zh", [D, n_tok], dt.bfloat16, kind="ExternalInput")
    zl = nc.dram_tensor("zl", [D, n_tok], dt.bfloat16, kind="ExternalInput")
    eh = nc.dram_tensor("eh", [D, n_codes], dt.bfloat16, kind="ExternalInput")
    el = nc.dram_tensor("el", [D, n_codes], dt.bfloat16, kind="ExternalInput")
    b3 = nc.dram_tensor("b3", [3, n_codes], dt.bfloat16, kind="ExternalInput")
    ones3 = nc.dram_tensor("ones3", [3, P], dt.bfloat16, kind="ExternalInput")
    iota = nc.dram_tensor("iota", [P, n_groups], dt.float32, kind="ExternalInput")
    cb = nc.dram_tensor("cb", [n_codes, D], dt.float32, kind="ExternalInput")
    out_e = nc.dram_tensor("out_e", [n_tok, D], dt.float32, kind="ExternalOutput")
    out_i = nc.dram_tensor("out_i", [n_tok, 1], dt.int32, kind="ExternalOutput")

    with tile.TileContext(nc) as tc:
        with (
            tc.tile_pool(name="const", bufs=1) as cpool,
            tc.tile_pool(name="stats", bufs=3) as spool,
            tc.tile_pool(name="emb", bufs=3) as epool,
            tc.tile_pool(name="psum", bufs=2, space="PSUM") as ppool,
        ):
            eh_t = cpool.tile([P, 2, n_codes], dt.bfloat16)
            nc.sync.dma_start(eh_t[:], eh.rearrange("(c p) n -> p c n", p=P))
            el_t = cpool.tile([P, 2, n_codes], dt.bfloat16)
            nc.sync.dma_start(el_t[:], el.rearrange("(c p) n -> p c n", p=P))
            b3_t = cpool.tile([3, n_codes], dt.bfloat16)
            nc.sync.dma_start(b3_t[:], b3[:])
            ones_t = cpool.tile([3, P], dt.bfloat16)
            nc.sync.dma_start(ones_t[:], ones3[:])
            iota_t = cpool.tile([P, n_groups], dt.float32)
            nc.sync.dma_start(iota_t[:], iota[:])
            zh_t = cpool.tile([P, 2, n_tok], dt.bfloat16)
            nc.sync.dma_start(zh_t[:], zh.rearrange("(c p) n -> p c n", p=P))
            zl_t = cpool.tile([P, 2, n_tok], dt.bfloat16)
            nc.sync.dma_start(zl_t[:], zl.rearrange("(c p) n -> p c n", p=P))

            for t in range(n_tiles):
                tok = slice(t * P, (t + 1) * P)
                gmax = spool.tile([P, n_groups, 8], dt.float32, tag="gmax")
                gidx = spool.tile([P, n_groups, 8], dt.uint32, tag="gidx")
                for g in range(n_groups):
                    ps = ppool.tile([P, gsize], dt.float32, tag="ps")
                    # per 512-col slice: bias starts the accumulation group,
                    # zl.eh chunk-1 ends it
                    passes = [
                        (ones_t[:], b3_t, True, False),
                        (zh_t[:, 0, tok], eh_t[:, 0], False, False),
                        (zh_t[:, 0, tok], el_t[:, 0], False, False),
                        (zh_t[:, 1, tok], eh_t[:, 1], False, False),
                        (zh_t[:, 1, tok], el_t[:, 1], False, False),
                        (zl_t[:, 0, tok], eh_t[:, 0], False, False),
                        (zl_t[:, 1, tok], eh_t[:, 1], False, True),
                    ]
                    for w, rhs, is_first, is_last in passes:
                        for c in range(nblk):
                            cols = slice(g * gsize + c * 512, g * gsize + (c + 1) * 512)
                            nc.tensor.matmul(
                                ps[:, c * 512 : (c + 1) * 512],
                                w,
                                rhs[:, cols],
                                start=is_first,
                                stop=is_last,
                            )
                    nc.vector.max(out=gmax[:, g, :], in_=ps[:])
                    nc.vector.max_index(gidx[:, g, :], gmax[:, g, :], ps[:])

                # combine the group winners (first-occurrence tie semantics)
                vpad = spool.tile([P, 8], dt.float32, tag="vpad")
                nc.vector.memset(vpad[:], -3.0e38)
                nc.vector.tensor_copy(vpad[:, 0:n_groups], gmax[:, :, 0])
                m8 = spool.tile([P, 8], dt.float32, tag="m8")
                nc.vector.max(out=m8[:], in_=vpad[:])
                g8 = spool.tile([P, 8], dt.uint32, tag="g8")
                nc.vector.max_index(g8[:], m8[:], vpad[:])

                gf = spool.tile([P, 1], dt.float32, tag="gf")
                nc.vector.tensor_copy(gf[:], g8[:, 0:1])
                i4f = spool.tile([P, n_groups], dt.float32, tag="i4f")
                nc.vector.tensor_copy(i4f[:], gidx[:, :, 0])
                eq = spool.tile([P, n_groups], dt.float32, tag="eq")
                nc.vector.tensor_scalar(
                    eq[:], iota_t[:], gf[:], None, op0=mybir.AluOpType.is_equal
                )
                nc.vector.tensor_tensor(
                    eq[:], eq[:], i4f[:], op=mybir.AluOpType.mult
                )
                loc = spool.tile([P, 1], dt.float32, tag="loc")
                nc.vector.reduce_sum(loc[:], eq[:], axis=mybir.AxisListType.X)
                idxf = spool.tile([P, 1], dt.float32, tag="idxf")
                nc.vector.tensor_scalar_mul(idxf[:], gf[:], float(gsize))
                nc.vector.tensor_tensor(
                    idxf[:], idxf[:], loc[:], op=mybir.AluOpType.add
                )
                idx_i = spool.tile([P, 1], dt.int32, tag="idxi")
                nc.vector.tensor_copy(idx_i[:], idxf[:])
                idx_u = spool.tile([P, 1], dt.uint32, tag="idxu")
                nc.vector.tensor_copy(idx_u[:], idxf[:])
                nc.sync.dma_start(out_i[tok, :], idx_i[:])

                emb_t = epool.tile([P, D], dt.float32, tag="emb")
                nc.gpsimd.indirect_dma_start(
                    out=emb_t[:],
                    out_offset=None,
                    in_=cb[:],
                    in_offset=bass.IndirectOffsetOnAxis(ap=idx_u[:, :1], axis=0),
                )
                nc.sync.dma_start(out_e[tok, :], emb_t[:])

    return nc


def _prep_inputs(z, codebook, n_tok_per_core, n_codes, gsize):
    """Host-side layout prep: token sharding, bf16 limb splits, bias limbs."""
    bf16 = ml_dtypes.bfloat16
    n_groups = n_codes // gsize

    zf = np.ascontiguousarray(np.asarray(z, np.float32).reshape(-1, D))
    cbf = np.ascontiguousarray(np.asarray(codebook, np.float32))

    cbT = np.ascontiguousarray(cbf.T)  # [D, K]
    eh = cbT.astype(bf16)
    el = (cbT - eh.astype(np.float32)).astype(bf16)
    bias = -0.5 * (cbf.astype(np.float64) ** 2).sum(axis=1)  # [K]
    b1 = bias.astype(np.float32).astype(bf16)
    r = bias - b1.astype(np.float64)
    b2 = r.astype(np.float32).astype(bf16)
    r = r - b2.astype(np.float64)
    b3l = r.astype(np.float32).astype(bf16)
    b3 = np.ascontiguousarray(np.stack([b1, b2, b3l], axis=0))  # [3, K]
    ones3 = np.ones((3, P), dtype=bf16)
    iota = np.broadcast_to(
        np.arange(n_groups, dtype=np.float32)[None, :], (P, n_groups)
    ).copy()

    shared = {
        "eh": np.ascontiguousarray(eh),
        "el": np.ascontiguousarray(el),
        "b3": b3,
        "ones3": ones3,
        "iota": iota,
        "cb": cbf,
    }
    in_maps = []
    ncores = zf.shape[0] // n_tok_per_core
    for c in range(ncores):
        zs = zf[c * n_tok_per_core : (c + 1) * n_tok_per_core]  # [n_tok, D]
        zT = np.ascontiguousarray(zs.T)  # [D, n_tok]
        zh = zT.astype(bf16)
        zl = (zT - zh.astype(np.float32)).astype(bf16)
        in_maps.append(
            {
                "zh": np.ascontiguousarray(zh),
                "zl": np.ascontiguousarray(zl),
                **shared,
            }
        )
    return in_maps


def _run(z, codebook, trace=False, trace_kwargs=None):
    from concourse.bass_utils import run_bass_kernel_spmd

    in_maps = _prep_inputs(z, codebook, TOK_PER_CORE, K, 2048)
    nc = _build(TOK_PER_CORE, K, 2048)
    nc.finalize()
    r = run_bass_kernel_spmd(
        nc,
        in_maps,
        core_ids=list(range(NCORES)),
        trace=trace,
        **(trace_kwargs or {}),
    )
    emb = np.concatenate([res["out_e"] for res in r.results], axis=0)
    idx = np.concatenate([res["out_i"] for res in r.results], axis=0)
    embedding = emb.reshape(B, S, D).astype(np.float32)
    q_indices = idx.reshape(B, S, 1).astype(np.int32)
    return (embedding, q_indices), r


def kernel(z, codebook):
    out, _ = _run(z, codebook, trace=False)
    return out


# revision 7
# speedup vs baseline: 1.0555x; 1.0555x over previous
"""VQ codebook kernel for 8 TRN2 NeuronCores.

reference math:
    dist  = ||z||^2 + ||e||^2 - 2 z.e       (argmin over 8192 codes)
    out   = (codebook[idx] (B,S,D) f32, idx (B,S,1) i32)

device math (argmin-equivalent, ||z||^2 dropped):
    score = z.e - ||e||^2/2                 (argmax)

Sharding: data-parallel on the flattened token dim (32768 tokens -> 4096
per core), codebook replicated.

Precision: the z.e GEMM runs as three bf16 passes (zh.eh + zh.el + zl.eh,
hi/lo bf16 limb split) which reproduces the fp32 reference argmin exactly
on this data (validated: 0/32768 flips; fp32r single-pass flips 8 tokens).
The -||e||^2/2 bias is folded into the same PSUM accumulation as a K=3
matmul of three bf16 bias limbs against a ones vector.

Argmax: per 2048-code PSUM group, DVE max (top-8) + max_index directly on
PSUM; the 4 group winners are combined with first-occurrence tie semantics
matching jnp.argmin. Embedding rows are gathered from DRAM by indirect DMA.
"""

import sys

sys.path.insert(0, "/opt/trn_rl_repo")

import numpy as np
import ml_dtypes

B, S, D, K = 8, 4096, 256, 8192
NCORES = 8
NTOK = B * S
TOK_PER_CORE = NTOK // NCORES
P = 128


_LDW_OPT = False  # walrus ldw-opt rejects Bacc's standalone InstLdweights;
# LDWEIGHTS is fully hidden behind the 512-col matmuls anyway (measured).


def _enable_ldw_opt():
    """Let walrus dedup back-to-back LDWEIGHTS with identical weight APs.

    Each 128-token weight tile feeds 4 consecutive 512-col matmuls; without
    the dedup every matmul re-loads its weights (~24 ns/matmul exposed on
    the PE stream, ~84 us over the kernel).
    """
    import concourse.bass_utils as bu

    if getattr(bu, "_vq_ldw_patched", False):
        return
    orig = bu.run_command

    def run_command_ldw(argv, **kwargs):
        argv = [
            "--enable-ldw-opt=true" if a == "--enable-ldw-opt=false" else a
            for a in argv
        ]
        return orig(argv, **kwargs)

    bu.run_command = run_command_ldw
    bu._vq_ldw_patched = True


def _build(n_tok, n_codes, gsize):
    import concourse.bacc as bacc
    import concourse.bass as bass
    import concourse.mybir as mybir
    import concourse.tile as tile

    dt = mybir.dt
    n_tiles = n_tok // P
    n_groups = n_codes // gsize
    nblk = gsize // 512
    assert n_groups <= 8

    nc = bacc.Bacc(trn_type="TRN2")
    zh = nc.dram_tensor("zh", [D, n_tok], dt.bfloat16, kind="ExternalInput")
    zl = nc.dram_tensor("zl", [D, n_tok], dt.bfloat16, kind="ExternalInput")
    eh = nc.dram_tensor("eh", [D, n_codes], dt.bfloat16, kind="ExternalInput")
    el = nc.dram_tensor("el", [D, n_codes], dt.bfloat16, kind="ExternalInput")
    b3 = nc.dram_tensor("b3", [P, n_codes], dt.bfloat16, kind="ExternalInput")
    ones3 = nc.dram_tensor("ones3", [P, P], dt.bfloat16, kind="ExternalInput")
    iota = nc.dram_tensor("iota", [P, n_groups], dt.float32, kind="ExternalInput")
    cb = nc.dram_tensor("cb", [n_codes, D], dt.float32, kind="ExternalInput")
    out_e = nc.dram_tensor("out_e", [n_tok, D], dt.float32, kind="ExternalOutput")
    out_i = nc.dram_tensor("out_i", [n_tok, 1], dt.int32, kind="ExternalOutput")

    with tile.TileContext(nc) as tc:
        with (
            tc.tile_pool(name="const", bufs=1) as cpool,
            tc.tile_pool(name="stats", bufs=3) as spool,
            tc.tile_pool(name="emb", bufs=3) as epool,
            tc.tile_pool(name="psum", bufs=2, space="PSUM") as ppool,
        ):
            # DMA order = first-use order; per-(chunk, group) codebook tiles
            # keep the dependency granularity fine so group-0 matmuls start
            # after ~6MB of preload instead of the full 12MB.
            ones_t = cpool.tile([P, P], dt.bfloat16)
            nc.sync.dma_start(ones_t[:], ones3[:])
            b3_t = cpool.tile([P, n_codes], dt.bfloat16)
            nc.sync.dma_start(b3_t[:], b3[:])
            zh_t = cpool.tile([P, 2, n_tok], dt.bfloat16)
            nc.sync.dma_start(zh_t[:], zh.rearrange("(c p) n -> p c n", p=P))
            zl_t = cpool.tile([P, 2, n_tok], dt.bfloat16)
            nc.sync.dma_start(zl_t[:], zl.rearrange("(c p) n -> p c n", p=P))
            eh_g = {}
            el_g = {}
            for g in range(n_groups):
                cols = slice(g * gsize, (g + 1) * gsize)
                for dc in range(2):
                    rows = slice(dc * P, (dc + 1) * P)
                    t_ = cpool.tile([P, gsize], dt.bfloat16, tag=f"eh_{g}_{dc}")
                    nc.sync.dma_start(t_[:], eh[rows, cols])
                    eh_g[(g, dc)] = t_
                    t_ = cpool.tile([P, gsize], dt.bfloat16, tag=f"el_{g}_{dc}")
                    nc.sync.dma_start(t_[:], el[rows, cols])
                    el_g[(g, dc)] = t_
            iota_t = cpool.tile([P, n_groups], dt.float32)
            nc.sync.dma_start(iota_t[:], iota[:])

            for t in range(n_tiles):
                tok = slice(t * P, (t + 1) * P)
                gmax = spool.tile([P, n_groups, 8], dt.float32, tag="gmax")
                gidx = spool.tile([P, n_groups, 8], dt.uint32, tag="gidx")
                for g in range(n_groups):
                    ps = ppool.tile([P, gsize], dt.float32, tag="ps")
                    # per 512-col slice: bias starts the accumulation group,
                    # zl.eh chunk-1 ends it
                    passes = [
                        (ones_t[:], b3_t[:, g * gsize : (g + 1) * gsize], True, False),
                        (zh_t[:, 0, tok], eh_g[(g, 0)], False, False),
                        (zh_t[:, 0, tok], el_g[(g, 0)], False, False),
                        (zh_t[:, 1, tok], eh_g[(g, 1)], False, False),
                        (zh_t[:, 1, tok], el_g[(g, 1)], False, False),
                        (zl_t[:, 0, tok], eh_g[(g, 0)], False, False),
                        (zl_t[:, 1, tok], eh_g[(g, 1)], False, True),
                    ]
                    for w, rhs, is_first, is_last in passes:
                        for c in range(nblk):
                            nc.tensor.matmul(
                                ps[:, c * 512 : (c + 1) * 512],
                                w,
                                rhs[:, c * 512 : (c + 1) * 512],
                                start=is_first,
                                stop=is_last,
                            )
                    nc.vector.max(out=gmax[:, g, :], in_=ps[:])
                    nc.vector.max_index(gidx[:, g, :], gmax[:, g, :], ps[:])

                # combine the group winners (first-occurrence tie semantics)
                vpad = spool.tile([P, 8], dt.float32, tag="vpad")
                nc.vector.memset(vpad[:], -3.0e38)
                nc.vector.tensor_copy(vpad[:, 0:n_groups], gmax[:, :, 0])
                m8 = spool.tile([P, 8], dt.float32, tag="m8")
                nc.vector.max(out=m8[:], in_=vpad[:])
                g8 = spool.tile([P, 8], dt.uint32, tag="g8")
                nc.vector.max_index(g8[:], m8[:], vpad[:])

                gf = spool.tile([P, 1], dt.float32, tag="gf")
                nc.vector.tensor_copy(gf[:], g8[:, 0:1])
                i4f = spool.tile([P, n_groups], dt.float32, tag="i4f")
                nc.vector.tensor_copy(i4f[:], gidx[:, :, 0])
                eq = spool.tile([P, n_groups], dt.float32, tag="eq")
                nc.vector.tensor_scalar(
                    eq[:], iota_t[:], gf[:], None, op0=mybir.AluOpType.is_equal
                )
                nc.vector.tensor_tensor(
                    eq[:], eq[:], i4f[:], op=mybir.AluOpType.mult
                )
                loc = spool.tile([P, 1], dt.float32, tag="loc")
                nc.vector.reduce_sum(loc[:], eq[:], axis=mybir.AxisListType.X)
                idxf = spool.tile([P, 1], dt.float32, tag="idxf")
                nc.vector.tensor_scalar_mul(idxf[:], gf[:], float(gsize))
                nc.vector.tensor_tensor(
                    idxf[:], idxf[:], loc[:], op=mybir.AluOpType.add
                )
                idx_i = spool.tile([P, 1], dt.int32, tag="idxi")
                nc.vector.tensor_copy(idx_i[:], idxf[:])
                idx_u = spool.tile([P, 1], dt.uint32, tag="idxu")
                nc.vector.tensor_copy(idx_u[:], idxf[:])
                nc.sync.dma_start(out_i[tok, :], idx_i[:])

                emb_t = epool.tile([P, D], dt.float32, tag="emb")
                nc.gpsimd.indirect_dma_start(
                    out=emb_t[:],
                    out_offset=None,
                    in_=cb[:],
                    in_offset=bass.IndirectOffsetOnAxis(ap=idx_u[:, :1], axis=0),
                )
                nc.sync.dma_start(out_e[tok, :], emb_t[:])

    return nc


def _prep_inputs(z, codebook, n_tok_per_core, n_codes, gsize):
    """Host-side layout prep: token sharding, bf16 limb splits, bias limbs."""
    bf16 = ml_dtypes.bfloat16
    n_groups = n_codes // gsize

    zf = np.ascontiguousarray(np.asarray(z, np.float32).reshape(-1, D))
    cbf = np.ascontiguousarray(np.asarray(codebook, np.float32))

    cbT = np.ascontiguousarray(cbf.T)  # [D, K]
    eh = cbT.astype(bf16)
    el = (cbT - eh.astype(np.float32)).astype(bf16)
    bias = -0.5 * (cbf.astype(np.float64) ** 2).sum(axis=1)  # [K]
    b1 = bias.astype(np.float32).astype(bf16)
    r = bias - b1.astype(np.float64)
    b2 = r.astype(np.float32).astype(bf16)
    r = r - b2.astype(np.float64)
    b3l = r.astype(np.float32).astype(bf16)
    b3 = np.zeros((P, len(bias)), dtype=bf16)  # K=128-padded bias limbs
    b3[0], b3[1], b3[2] = b1, b2, b3l
    ones3 = np.ones((P, P), dtype=bf16)
    iota = np.broadcast_to(
        np.arange(n_groups, dtype=np.float32)[None, :], (P, n_groups)
    ).copy()

    shared = {
        "eh": np.ascontiguousarray(eh),
        "el": np.ascontiguousarray(el),
        "b3": b3,
        "ones3": ones3,
        "iota": iota,
        "cb": cbf,
    }
    in_maps = []
    ncores = zf.shape[0] // n_tok_per_core
    for c in range(ncores):
        zs = zf[c * n_tok_per_core : (c + 1) * n_tok_per_core]  # [n_tok, D]
        zT = np.ascontiguousarray(zs.T)  # [D, n_tok]
        zh = zT.astype(bf16)
        zl = (zT - zh.astype(np.float32)).astype(bf16)
        in_maps.append(
            {
                "zh": np.ascontiguousarray(zh),
                "zl": np.ascontiguousarray(zl),
                **shared,
            }
        )
    return in_maps


def _run(z, codebook, trace=False, trace_kwargs=None):
    from concourse.bass_utils import run_bass_kernel_spmd

    if _LDW_OPT:
        _enable_ldw_opt()
    in_maps = _prep_inputs(z, codebook, TOK_PER_CORE, K, 2048)
    nc = _build(TOK_PER_CORE, K, 2048)
    nc.finalize()
    r = run_bass_kernel_spmd(
        nc,
        in_maps,
        core_ids=list(range(NCORES)),
        trace=trace,
        **(trace_kwargs or {}),
    )
    emb = np.concatenate([res["out_e"] for res in r.results], axis=0)
    idx = np.concatenate([res["out_i"] for res in r.results], axis=0)
    embedding = emb.reshape(B, S, D).astype(np.float32)
    q_indices = idx.reshape(B, S, 1).astype(np.int32)
    return (embedding, q_indices), r


def kernel(z, codebook):
    out, _ = _run(z, codebook, trace=False)
    return out
